# revision 13
# baseline (speedup 1.0000x reference)
"""Distributed multi-head attention kernel for 8 TRN2 NeuronCores.

Sharding: tensor-parallel over heads (2 heads/core) for the qkv projection
and attention; AllToAll exchange of the attention output; row-parallel output
projection (each core produces a transposed 512-row slice of the final
output); host reassembles.

Compute in bf16 on the PE array (f32 PSUM accumulation everywhere, f32
softmax denominators/normalization). The host pre-transposes x to [dim, b*s]
and pre-casts x/wqkv/wo to bf16 as part of sharding/layout prep.
"""

import sys

sys.path.insert(0, "/opt/trn_rl_repo")

import ml_dtypes
import numpy as np

# Problem constants (hardcoded per harness contract)
B = 2
S = 2048
DIM = 1024
N_HEAD = 16
HD = 64  # head dim
SCALE = HD ** (-0.5)
R = B * S  # 4096 flattened rows
NCORES = 8
HPC = N_HEAD // NCORES  # 2 heads per core
FPC = HPC * HD  # 128 features per core
RPC = R // NCORES  # 512 rows per core (output row slice)

KT = DIM // 128  # 8 k-tiles over the model dim
NKT = S // 128  # 16 key tiles per sequence
NQB = S // 512  # 4 query blocks per sequence

_CACHED = {}


def _build_graph():
    import concourse.mybir as mybir
    import concourse.tile as tile
    from concourse import bacc
    from concourse.masks import make_identity

    f32 = mybir.dt.float32
    f32r = mybir.dt.float32r
    bf16 = mybir.dt.bfloat16
    EXP = mybir.ActivationFunctionType.Exp
    RECIP = mybir.ActivationFunctionType.Reciprocal

    nc = bacc.Bacc(
        "TRN2",
        target_bir_lowering=False,
        debug=False,
        num_devices=NCORES,
    )

    # xt: x pre-transposed+bf16 on host -> [dim, b*s]
    xt = nc.dram_tensor("xt", [DIM, R], bf16, kind="ExternalInput").ap()
    wqkv = nc.dram_tensor("wqkv", [DIM, 3 * FPC], bf16, kind="ExternalInput").ap()
    bqkv = nc.dram_tensor("bqkv", [3, FPC], f32, kind="ExternalInput").ap()
    wo = nc.dram_tensor("wo", [DIM, DIM], bf16, kind="ExternalInput").ap()
    bo = nc.dram_tensor("bo", [8, 128], f32, kind="ExternalInput").ap()
    out = nc.dram_tensor("out", [DIM, RPC], f32, kind="ExternalOutput").ap()

    with tile.TileContext(nc) as tc:
        with tc.tile_pool(name="glob", bufs=1) as glob:
            ident16 = glob.tile([128, 128], bf16)
            make_identity(nc, ident16)
            # biases as [128, 1] per-partition vectors (f32)
            bias_qkv = glob.tile([128, 3], f32)
            for m in range(3):
                nc.gpsimd.dma_start(out=bias_qkv[:, m : m + 1], in_=bqkv[m : m + 1, :])
            bias_o = glob.tile([128, 8], f32)
            for m in range(8):
                nc.gpsimd.dma_start(out=bias_o[:, m : m + 1], in_=bo[m : m + 1, :])

            # persistent activations (bf16 compute operands)
            qT = glob.tile([128, R], bf16)  # [2 heads x 64 d, b*s]
            kT = glob.tile([128, R], bf16)
            # v natural layout + ones column: per (b, h, kt) a [128, 65] block
            v_nat = glob.tile([128, B * HPC * NKT * 65], bf16)
            ones_tmp = glob.tile([128, 64], f32)
            nc.vector.memset(ones_tmp[:], 1.0)
            nc.vector.tensor_copy(out=v_nat[:, 64::65], in_=ones_tmp[:])
            # attention output, transposed, bf16, one tile per head
            oT = [glob.tile([64, R], bf16, name=f"oT_{hh}") for hh in range(HPC)]

            # ---------------- stage A: qkv projection ---------------------
            with (
                tc.tile_pool(name="xTp", bufs=2) as xT_pool,
                tc.tile_pool(name="wq", bufs=1) as wq_pool,
                tc.tile_pool(name="vt", bufs=2) as vt_pool,
                tc.tile_pool(name="ps_tr", bufs=2, space="PSUM") as ps_tr,
                tc.tile_pool(name="ps_proj", bufs=1, space="PSUM") as ps_proj,
            ):
                wqkv_sb = []
                for k in range(KT):
                    w_t = wq_pool.tile([128, 3 * FPC], bf16, name=f"w_{k}")
                    nc.gpsimd.dma_start(
                        out=w_t[:], in_=wqkv[k * 128 : (k + 1) * 128, :]
                    )
                    wqkv_sb.append(w_t)

                NG = 4  # row groups of 1024
                for g in range(NG):
                    xT_g = [
                        xT_pool.tile([128, 1024], bf16, name=f"xT_{k}", tag=f"xT{k}")
                        for k in range(KT)
                    ]
                    for k in range(KT):
                        nc.sync.dma_start(
                            out=xT_g[k][:],
                            in_=xt[k * 128 : (k + 1) * 128, g * 1024 : (g + 1) * 1024],
                        )
                    # projection for this row group; 6 psum banks (q/k/v x 2)
                    pp = [
                        [
                            ps_proj.tile(
                                [128, 512], f32, name=f"pp_{m}_{h}", tag=f"pp{m}{h}"
                            )
                            for h in range(2)
                        ]
                        for m in range(3)
                    ]
                    for k in range(KT):
                        for m in range(3):
                            for h in range(2):
                                nc.tensor.matmul(
                                    pp[m][h][:],
                                    lhsT=wqkv_sb[k][:, m * 128 : (m + 1) * 128],
                                    rhs=xT_g[k][:, h * 512 : (h + 1) * 512],
                                    start=(k == 0),
                                    stop=(k == KT - 1),
                                )
                    for h in range(2):
                        col0 = g * 1024 + h * 512
                        nc.vector.tensor_scalar_add(
                            out=qT[:, col0 : col0 + 512],
                            in0=pp[0][h][:],
                            scalar1=bias_qkv[:, 0:1],
                        )
                        nc.vector.tensor_scalar_add(
                            out=kT[:, col0 : col0 + 512],
                            in0=pp[1][h][:],
                            scalar1=bias_qkv[:, 1:2],
                        )
                        # v: bias (-> bf16), then transpose to natural layout
                        vT_blk = vt_pool.tile(
                            [128, 512], bf16, name="vT_blk", tag="vT_blk"
                        )
                        nc.vector.tensor_scalar_add(
                            out=vT_blk[:], in0=pp[2][h][:], scalar1=bias_qkv[:, 2:3]
                        )
                        for j in range(4):
                            row0 = col0 + j * 128
                            b_idx = row0 // S
                            kt = (row0 % S) // 128
                            for hh in range(HPC):
                                pstv = ps_tr.tile(
                                    [128, 64], bf16, name="pstv", tag="pstv"
                                )
                                nc.tensor.transpose(
                                    pstv[:],
                                    vT_blk[
                                        hh * 64 : (hh + 1) * 64,
                                        j * 128 : (j + 1) * 128,
                                    ],
                                    ident16[
                                        hh * 64 : (hh + 1) * 64,
                                        hh * 64 : (hh + 1) * 64,
                                    ],
                                )
                                col = ((b_idx * HPC + hh) * NKT + kt) * 65
                                nc.vector.tensor_copy(
                                    out=v_nat[:, col : col + 64], in_=pstv[:]
                                )

            # ---------------- stage C: attention --------------------------
            with (
                tc.tile_pool(name="ps_st", bufs=2, space="PSUM") as ps_st,
                tc.tile_pool(name="ps_o", bufs=2, space="PSUM") as ps_o,
                tc.tile_pool(name="ptp", bufs=3) as pt_pool,
                tc.tile_pool(name="nrm", bufs=2) as nrm_pool,
                tc.tile_pool(name="onesp", bufs=1) as ones_pool,
            ):
                # ones row living on partition 64 (lane-aligned with po row 64)
                ones64 = ones_pool.tile([65, 64], f32r)
                nc.vector.tensor_copy(out=ones64[64:65, :], in_=ones_tmp[64:65, :])
                for b in range(B):
                    for qb in range(NQB):
                        q0 = b * S + qb * 512
                        po = ps_o.tile([65, 1024], f32, name="po", tag="po")
                        for kt in range(NKT):
                            k0 = b * S + kt * 128
                            # both heads' S.T blocks into one 2-bank psum
                            # tile; explicit row-strip tile_position so the
                            # two K=64 matmuls run concurrently on the array
                            pst = ps_st.tile([128, 1024], f32, name="st", tag="st")
                            for hh in range(HPC):
                                nc.tensor.matmul(
                                    pst[:, hh * 512 : (hh + 1) * 512],
                                    lhsT=kT[hh * 64 : (hh + 1) * 64, k0 : k0 + 128],
                                    rhs=qT[hh * 64 : (hh + 1) * 64, q0 : q0 + 512],
                                    start=True,
                                    stop=True,
                                    tile_position=(hh * 64, 0),
                                )
                            ptile = pt_pool.tile(
                                [128, 1024], bf16, name="ptile", tag="ptile"
                            )
                            nc.scalar.activation(ptile[:], pst[:], EXP, scale=SCALE)
                            for hh in range(HPC):
                                col = ((b * HPC + hh) * NKT + kt) * 65
                                nc.tensor.matmul(
                                    po[:, hh * 512 : (hh + 1) * 512],
                                    lhsT=v_nat[:, col : col + 65],
                                    rhs=ptile[:, hh * 512 : (hh + 1) * 512],
                                    start=(kt == 0),
                                    stop=(kt == NKT - 1),
                                )
                        # one reciprocal over both heads' denominators, then
                        # per-head PE broadcast to 64 partitions (psum slots
                        # tag-shared with the S tiles) and normalize (bf16).
                        sden = nrm_pool.tile([65, 1024], f32r, name="sden", tag="sden")
                        with nc.allow_low_precision(reason="softmax denom"):
                            nc.vector.reciprocal(sden[64:65, :], po[64:65, :])
                        for hh in range(HPC):
                            pbc = ps_st.tile([64, 512], f32, name="pbc", tag="st")
                            nc.tensor.matmul(
                                pbc[:],
                                lhsT=ones64[64:65, :],
                                rhs=sden[64 : 65, hh * 512 : (hh + 1) * 512],
                                start=True,
                                stop=True,
                            )
                            bcs = nrm_pool.tile([64, 512], f32, name="bcs", tag="bcs")
                            nc.vector.tensor_copy(out=bcs[:], in_=pbc[:])
                            nc.vector.tensor_mul(
                                out=oT[hh][:, q0 : q0 + 512],
                                in0=po[0:64, hh * 512 : (hh + 1) * 512],
                                in1=bcs[:],
                            )

            # ---------------- stage D: exchange + out projection ----------
            with (
                tc.tile_pool(name="dram", bufs=1, space="DRAM") as dram_pool,
                tc.tile_pool(name="wosb", bufs=1) as wo_pool,
                tc.tile_pool(name="ots", bufs=1) as ots_pool,
                tc.tile_pool(name="psout", bufs=1, space="PSUM") as ps_out,
                tc.tile_pool(name="outt", bufs=2) as out_pool,
            ):
                wo_sb = []
                for k in range(KT):
                    w_t = wo_pool.tile([128, DIM], bf16, name=f"wo_{k}")
                    nc.gpsimd.dma_start(
                        out=w_t[:], in_=wo[k * 128 : (k + 1) * 128, :]
                    )
                    wo_sb.append(w_t)

                a2a_in = dram_pool.tile([DIM, RPC], bf16, name="a2a_in")
                a2a_out = dram_pool.tile([DIM, RPC], bf16, name="a2a_out")
                for c in range(NCORES):
                    for hh in range(HPC):
                        nc.sync.dma_start(
                            out=a2a_in[c * 128 + hh * 64 : c * 128 + (hh + 1) * 64, :],
                            in_=oT[hh][:, c * RPC : (c + 1) * RPC],
                        )
                nc.gpsimd.collective_compute(
                    "AllToAll",
                    mybir.AluOpType.bypass,
                    replica_groups=[list(range(NCORES))],
                    ins=[a2a_in[:].opt()],
                    outs=[a2a_out[:].opt()],
                )
                oTs = []
                for k in range(KT):
                    o_t = ots_pool.tile([128, RPC], bf16, name=f"oTs_{k}")
                    nc.sync.dma_start(
                        out=o_t[:], in_=a2a_out[k * 128 : (k + 1) * 128, :]
                    )
                    oTs.append(o_t)
                pout = [
                    ps_out.tile([128, 512], f32, name=f"pout_{m}") for m in range(8)
                ]
                for k in range(KT):
                    for m in range(8):
                        nc.tensor.matmul(
                            pout[m][:],
                            lhsT=wo_sb[k][:, m * 128 : (m + 1) * 128],
                            rhs=oTs[k][:],
                            start=(k == 0),
                            stop=(k == KT - 1),
                        )
                for m in range(8):
                    o_sb = out_pool.tile([128, 512], f32, name="o_sb", tag="o_sb")
                    nc.vector.tensor_scalar_add(
                        out=o_sb[:], in0=pout[m][:], scalar1=bias_o[:, m : m + 1]
                    )
                    nc.sync.dma_start(out=out[m * 128 : (m + 1) * 128, :], in_=o_sb[:])

    nc.compile()
    return nc


def _get_graph():
    if "nc" not in _CACHED:
        _CACHED["nc"] = _build_graph()
    return _CACHED["nc"]


def _make_in_maps(x, wqkv, bqkv, wo, bo):
    bf = ml_dtypes.bfloat16
    x2 = np.asarray(x, dtype=np.float32).reshape(R, DIM)
    xt = np.ascontiguousarray(x2.T.astype(bf))  # [dim, b*s] bf16
    wqkv = np.asarray(wqkv, dtype=np.float32)
    bqkv = np.asarray(bqkv, dtype=np.float32)
    wo16 = np.ascontiguousarray(np.asarray(wo, dtype=np.float32).astype(bf))
    bo_f = np.ascontiguousarray(np.asarray(bo, dtype=np.float32).reshape(8, 128))

    in_maps = []
    for c in range(NCORES):
        w_s = np.ascontiguousarray(
            np.concatenate(
                [
                    wqkv[:, c * FPC : (c + 1) * FPC],
                    wqkv[:, DIM + c * FPC : DIM + (c + 1) * FPC],
                    wqkv[:, 2 * DIM + c * FPC : 2 * DIM + (c + 1) * FPC],
                ],
                axis=1,
            ).astype(bf)
        )
        b_s = np.ascontiguousarray(
            np.stack(
                [
                    bqkv[c * FPC : (c + 1) * FPC],
                    bqkv[DIM + c * FPC : DIM + (c + 1) * FPC],
                    bqkv[2 * DIM + c * FPC : 2 * DIM + (c + 1) * FPC],
                ],
                axis=0,
            )
        )
        in_maps.append({"xt": xt, "wqkv": w_s, "bqkv": b_s, "wo": wo16, "bo": bo_f})
    return in_maps


def kernel(x, wqkv, bqkv, wo, bo):
    from concourse.bass_utils import run_bass_kernel_spmd

    nc = _get_graph()
    in_maps = _make_in_maps(x, wqkv, bqkv, wo, bo)
    res = run_bass_kernel_spmd(nc, in_maps, core_ids=list(range(NCORES)))
    outs = [res.results[c]["out"] for c in range(NCORES)]  # each [1024, 512]
    full = np.concatenate([o.T for o in outs], axis=0)  # [4096, 1024]
    return np.ascontiguousarray(full.reshape(B, S, DIM)).astype(np.float32)


# revision 15
# speedup vs baseline: 1.0532x; 1.0532x over previous
"""Distributed multi-head attention kernel for 8 TRN2 NeuronCores.

Sharding: tensor-parallel over heads (2 heads/core) for the qkv projection
and attention; AllToAll exchange of the attention output; row-parallel output
projection (each core produces a transposed 512-row slice of the final
output); host reassembles.

Compute in bf16 on the PE array (f32 PSUM accumulation everywhere, f32
softmax denominators/normalization). The host pre-transposes x to [dim, b*s]
and pre-casts x/wqkv/wo to bf16 as part of sharding/layout prep.
"""

import sys

sys.path.insert(0, "/opt/trn_rl_repo")

import ml_dtypes
import numpy as np

# Problem constants (hardcoded per harness contract)
B = 2
S = 2048
DIM = 1024
N_HEAD = 16
HD = 64  # head dim
SCALE = HD ** (-0.5)
R = B * S  # 4096 flattened rows
NCORES = 8
HPC = N_HEAD // NCORES  # 2 heads per core
FPC = HPC * HD  # 128 features per core
RPC = R // NCORES  # 512 rows per core (output row slice)

KT = DIM // 128  # 8 k-tiles over the model dim
NKT = S // 128  # 16 key tiles per sequence
NQB = S // 512  # 4 query blocks per sequence

_CACHED = {}


def _build_graph():
    import concourse.mybir as mybir
    import concourse.tile as tile
    from concourse import bacc
    from concourse.masks import make_identity

    f32 = mybir.dt.float32
    f32r = mybir.dt.float32r
    bf16 = mybir.dt.bfloat16
    EXP = mybir.ActivationFunctionType.Exp
    LN = mybir.ActivationFunctionType.Ln

    nc = bacc.Bacc(
        "TRN2",
        target_bir_lowering=False,
        debug=False,
        num_devices=NCORES,
    )

    # xt: x pre-transposed+bf16 on host -> [dim, b*s]
    xt = nc.dram_tensor("xt", [DIM, R], bf16, kind="ExternalInput").ap()
    wqkv = nc.dram_tensor("wqkv", [DIM, 3 * FPC], bf16, kind="ExternalInput").ap()
    bqkv = nc.dram_tensor("bqkv", [3, FPC], f32, kind="ExternalInput").ap()
    wo = nc.dram_tensor("wo", [DIM, DIM], bf16, kind="ExternalInput").ap()
    bo = nc.dram_tensor("bo", [8, 128], f32, kind="ExternalInput").ap()
    out = nc.dram_tensor("out", [DIM, RPC], f32, kind="ExternalOutput").ap()

    with tile.TileContext(nc) as tc:
        with tc.tile_pool(name="glob", bufs=1) as glob:
            ident16 = glob.tile([128, 128], bf16)
            make_identity(nc, ident16)
            # biases as [128, 1] per-partition vectors (f32)
            bias_qkv = glob.tile([128, 3], f32)
            for m in range(3):
                nc.gpsimd.dma_start(out=bias_qkv[:, m : m + 1], in_=bqkv[m : m + 1, :])
            bias_o = glob.tile([128, 8], f32)
            for m in range(8):
                nc.gpsimd.dma_start(out=bias_o[:, m : m + 1], in_=bo[m : m + 1, :])

            # persistent activations (bf16 compute operands)
            qT = glob.tile([128, R], bf16)  # [2 heads x 64 d, b*s]
            kT = glob.tile([128, R], bf16)
            # v natural layout + ones column: per (b, h, kt) a [128, 65] block
            v_nat = glob.tile([128, B * HPC * NKT * 65], bf16)
            ones_tmp = glob.tile([128, 64], f32)
            nc.vector.memset(ones_tmp[:], 1.0)
            nc.vector.tensor_copy(out=v_nat[:, 64::65], in_=ones_tmp[:])
            # attention output, transposed, bf16, one tile per head
            oT = [glob.tile([64, R], bf16, name=f"oT_{hh}") for hh in range(HPC)]

            # ---------------- stage A: qkv projection ---------------------
            with (
                tc.tile_pool(name="xTp", bufs=2) as xT_pool,
                tc.tile_pool(name="wq", bufs=1) as wq_pool,
                tc.tile_pool(name="vt", bufs=2) as vt_pool,
                tc.tile_pool(name="ps_tr", bufs=2, space="PSUM") as ps_tr,
                tc.tile_pool(name="ps_proj", bufs=1, space="PSUM") as ps_proj,
            ):
                wqkv_sb = []
                for k in range(KT):
                    w_t = wq_pool.tile([128, 3 * FPC], bf16, name=f"w_{k}")
                    nc.gpsimd.dma_start(
                        out=w_t[:], in_=wqkv[k * 128 : (k + 1) * 128, :]
                    )
                    wqkv_sb.append(w_t)

                NG = 4  # row groups of 1024
                for g in range(NG):
                    xT_g = [
                        xT_pool.tile([128, 1024], bf16, name=f"xT_{k}", tag=f"xT{k}")
                        for k in range(KT)
                    ]
                    for k in range(KT):
                        nc.sync.dma_start(
                            out=xT_g[k][:],
                            in_=xt[k * 128 : (k + 1) * 128, g * 1024 : (g + 1) * 1024],
                        )
                    # projection for this row group; 6 psum banks (q/k/v x 2)
                    pp = [
                        [
                            ps_proj.tile(
                                [128, 512], f32, name=f"pp_{m}_{h}", tag=f"pp{m}{h}"
                            )
                            for h in range(2)
                        ]
                        for m in range(3)
                    ]
                    for k in range(KT):
                        for m in range(3):
                            for h in range(2):
                                nc.tensor.matmul(
                                    pp[m][h][:],
                                    lhsT=wqkv_sb[k][:, m * 128 : (m + 1) * 128],
                                    rhs=xT_g[k][:, h * 512 : (h + 1) * 512],
                                    start=(k == 0),
                                    stop=(k == KT - 1),
                                )
                    for h in range(2):
                        col0 = g * 1024 + h * 512
                        nc.vector.tensor_scalar_add(
                            out=qT[:, col0 : col0 + 512],
                            in0=pp[0][h][:],
                            scalar1=bias_qkv[:, 0:1],
                        )
                        nc.vector.tensor_scalar_add(
                            out=kT[:, col0 : col0 + 512],
                            in0=pp[1][h][:],
                            scalar1=bias_qkv[:, 1:2],
                        )
                        # v: bias (-> bf16), then transpose to natural layout
                        vT_blk = vt_pool.tile(
                            [128, 512], bf16, name="vT_blk", tag="vT_blk"
                        )
                        nc.vector.tensor_scalar_add(
                            out=vT_blk[:], in0=pp[2][h][:], scalar1=bias_qkv[:, 2:3]
                        )
                        for j in range(4):
                            row0 = col0 + j * 128
                            b_idx = row0 // S
                            kt = (row0 % S) // 128
                            for hh in range(HPC):
                                pstv = ps_tr.tile(
                                    [128, 64], bf16, name="pstv", tag="pstv"
                                )
                                nc.tensor.transpose(
                                    pstv[:],
                                    vT_blk[
                                        hh * 64 : (hh + 1) * 64,
                                        j * 128 : (j + 1) * 128,
                                    ],
                                    ident16[
                                        hh * 64 : (hh + 1) * 64,
                                        hh * 64 : (hh + 1) * 64,
                                    ],
                                )
                                col = ((b_idx * HPC + hh) * NKT + kt) * 65
                                nc.vector.tensor_copy(
                                    out=v_nat[:, col : col + 64], in_=pstv[:]
                                )

            # ---------------- stage C: attention --------------------------
            with (
                tc.tile_pool(name="ps_st", bufs=2, space="PSUM") as ps_st,
                tc.tile_pool(name="ps_o", bufs=2, space="PSUM") as ps_o,
                tc.tile_pool(name="ptp", bufs=3) as pt_pool,
                tc.tile_pool(name="nrm", bufs=2) as nrm_pool,
                tc.tile_pool(name="onesp", bufs=1) as ones_pool,
            ):
                # ones row living on partition 64 (lane-aligned with po row 64)
                ones64 = ones_pool.tile([65, 64], f32r)
                nc.vector.tensor_copy(out=ones64[64:65, :], in_=ones_tmp[64:65, :])
                for b in range(B):
                    for qb in range(NQB):
                        q0 = b * S + qb * 512
                        po = ps_o.tile([65, 1024], f32, name="po", tag="po")
                        for kt in range(NKT):
                            k0 = b * S + kt * 128
                            # both heads' S.T blocks into one 2-bank psum
                            # tile; explicit row-strip tile_position so the
                            # two K=64 matmuls run concurrently on the array
                            pst = ps_st.tile([128, 1024], f32, name="st", tag="st")
                            for hh in range(HPC):
                                nc.tensor.matmul(
                                    pst[:, hh * 512 : (hh + 1) * 512],
                                    lhsT=kT[hh * 64 : (hh + 1) * 64, k0 : k0 + 128],
                                    rhs=qT[hh * 64 : (hh + 1) * 64, q0 : q0 + 512],
                                    start=True,
                                    stop=True,
                                    tile_position=(hh * 64, 0),
                                )
                            ptile = pt_pool.tile(
                                [128, 1024], bf16, name="ptile", tag="ptile"
                            )
                            nc.scalar.activation(ptile[:], pst[:], EXP, scale=SCALE)
                            for hh in range(HPC):
                                col = ((b * HPC + hh) * NKT + kt) * 65
                                nc.tensor.matmul(
                                    po[:, hh * 512 : (hh + 1) * 512],
                                    lhsT=v_nat[:, col : col + 65],
                                    rhs=ptile[:, hh * 512 : (hh + 1) * 512],
                                    start=(kt == 0),
                                    stop=(kt == NKT - 1),
                                )
                        # one reciprocal over both heads' denominators, then
                        # per-head PE broadcast to 64 partitions (psum slots
                        # tag-shared with the S tiles) and normalize (bf16).
                        # reciprocal of the denominators via exp(-ln(d)) on
                        # ACT (1 elem/cycle/lane; DVE reciprocal is ~9x
                        # slower and stalls the PE at block boundaries)
                        lden = nrm_pool.tile([65, 1024], f32, name="lden", tag="lden")
                        nc.scalar.activation(
                            lden[64:65, :], po[64:65, :], LN
                        )
                        sden = nrm_pool.tile([65, 1024], f32r, name="sden", tag="sden")
                        with nc.allow_low_precision(reason="softmax denom"):
                            nc.scalar.activation(
                                sden[64:65, :], lden[64:65, :], EXP, scale=-1.0
                            )
                        for hh in range(HPC):
                            pbc = ps_st.tile([64, 512], f32, name="pbc", tag="st")
                            nc.tensor.matmul(
                                pbc[:],
                                lhsT=ones64[64:65, :],
                                rhs=sden[64 : 65, hh * 512 : (hh + 1) * 512],
                                start=True,
                                stop=True,
                            )
                            bcs = nrm_pool.tile([64, 512], f32, name="bcs", tag="bcs")
                            nc.vector.tensor_copy(out=bcs[:], in_=pbc[:])
                            nc.vector.tensor_mul(
                                out=oT[hh][:, q0 : q0 + 512],
                                in0=po[0:64, hh * 512 : (hh + 1) * 512],
                                in1=bcs[:],
                            )

            # ---------------- stage D: exchange + out projection ----------
            with (
                tc.tile_pool(name="dram", bufs=1, space="DRAM") as dram_pool,
                tc.tile_pool(name="wosb", bufs=1) as wo_pool,
                tc.tile_pool(name="ots", bufs=1) as ots_pool,
                tc.tile_pool(name="psout", bufs=1, space="PSUM") as ps_out,
                tc.tile_pool(name="outt", bufs=2) as out_pool,
            ):
                wo_sb = []
                for k in range(KT):
                    w_t = wo_pool.tile([128, DIM], bf16, name=f"wo_{k}")
                    nc.gpsimd.dma_start(
                        out=w_t[:], in_=wo[k * 128 : (k + 1) * 128, :]
                    )
                    wo_sb.append(w_t)

                a2a_in = dram_pool.tile([DIM, RPC], bf16, name="a2a_in")
                a2a_out = dram_pool.tile([DIM, RPC], bf16, name="a2a_out")
                for c in range(NCORES):
                    for hh in range(HPC):
                        nc.sync.dma_start(
                            out=a2a_in[c * 128 + hh * 64 : c * 128 + (hh + 1) * 64, :],
                            in_=oT[hh][:, c * RPC : (c + 1) * RPC],
                        )
                nc.gpsimd.collective_compute(
                    "AllToAll",
                    mybir.AluOpType.bypass,
                    replica_groups=[list(range(NCORES))],
                    ins=[a2a_in[:].opt()],
                    outs=[a2a_out[:].opt()],
                )
                oTs = []
                for k in range(KT):
                    o_t = ots_pool.tile([128, RPC], bf16, name=f"oTs_{k}")
                    nc.sync.dma_start(
                        out=o_t[:], in_=a2a_out[k * 128 : (k + 1) * 128, :]
                    )
                    oTs.append(o_t)
                pout = [
                    ps_out.tile([128, 512], f32, name=f"pout_{m}") for m in range(8)
                ]
                for k in range(KT):
                    for m in range(8):
                        nc.tensor.matmul(
                            pout[m][:],
                            lhsT=wo_sb[k][:, m * 128 : (m + 1) * 128],
                            rhs=oTs[k][:],
                            start=(k == 0),
                            stop=(k == KT - 1),
                        )
                for m in range(8):
                    o_sb = out_pool.tile([128, 512], f32, name="o_sb", tag="o_sb")
                    nc.vector.tensor_scalar_add(
                        out=o_sb[:], in0=pout[m][:], scalar1=bias_o[:, m : m + 1]
                    )
                    nc.sync.dma_start(out=out[m * 128 : (m + 1) * 128, :], in_=o_sb[:])

    nc.compile()
    return nc


def _get_graph():
    if "nc" not in _CACHED:
        _CACHED["nc"] = _build_graph()
    return _CACHED["nc"]


def _make_in_maps(x, wqkv, bqkv, wo, bo):
    bf = ml_dtypes.bfloat16
    x2 = np.asarray(x, dtype=np.float32).reshape(R, DIM)
    xt = np.ascontiguousarray(x2.T.astype(bf))  # [dim, b*s] bf16
    wqkv = np.asarray(wqkv, dtype=np.float32)
    bqkv = np.asarray(bqkv, dtype=np.float32)
    wo16 = np.ascontiguousarray(np.asarray(wo, dtype=np.float32).astype(bf))
    bo_f = np.ascontiguousarray(np.asarray(bo, dtype=np.float32).reshape(8, 128))

    in_maps = []
    for c in range(NCORES):
        w_s = np.ascontiguousarray(
            np.concatenate(
                [
                    wqkv[:, c * FPC : (c + 1) * FPC],
                    wqkv[:, DIM + c * FPC : DIM + (c + 1) * FPC],
                    wqkv[:, 2 * DIM + c * FPC : 2 * DIM + (c + 1) * FPC],
                ],
                axis=1,
            ).astype(bf)
        )
        b_s = np.ascontiguousarray(
            np.stack(
                [
                    bqkv[c * FPC : (c + 1) * FPC],
                    bqkv[DIM + c * FPC : DIM + (c + 1) * FPC],
                    bqkv[2 * DIM + c * FPC : 2 * DIM + (c + 1) * FPC],
                ],
                axis=0,
            )
        )
        in_maps.append({"xt": xt, "wqkv": w_s, "bqkv": b_s, "wo": wo16, "bo": bo_f})
    return in_maps


def kernel(x, wqkv, bqkv, wo, bo):
    from concourse.bass_utils import run_bass_kernel_spmd

    nc = _get_graph()
    in_maps = _make_in_maps(x, wqkv, bqkv, wo, bo)
    res = run_bass_kernel_spmd(nc, in_maps, core_ids=list(range(NCORES)))
    outs = [res.results[c]["out"] for c in range(NCORES)]  # each [1024, 512]
    full = np.concatenate([o.T for o in outs], axis=0)  # [4096, 1024]
    return np.ascontiguousarray(full.reshape(B, S, DIM)).astype(np.float32)
